# revision 10
# baseline (speedup 1.0000x reference)
"""nn_CrossAttention Trainium2 kernel — 8-core SPMD Bass/Tile implementation.

Sharding: core p -> batch b = p//2, query-row half h = p%2 (data parallel over
B=4, sequence-parallel over TN within each batch pair).

Per-core dataflow (v3):
  tT,xT   PE-transpose of the inputs; four 128x128 transposes share one
          [128,512] PSUM tile so a single wide copy drains them (4x fewer
          PSUM->SBUF copies)
  qT,kT   projections in channel-major layout (f32r = tf32-class precision)
  v       projection in natural row-major layout (bf16)
  sT      score tiles computed *transposed* (keys on partitions, queries free)
          two key-blocks share one [128,1024] PSUM tile so a single 1024-wide
          ScalarE exp serves both (amortizes the +352-cycle ACT overhead)
  D       softmax denominators via ones-vector matmul (PE partition reduce)
  oT      v.T @ eT accumulated over keys, normalized by 1/D broadcast, bf16
  The reference's "transpose(1,2).reshape" permutation assigns each core a
  contiguous 192-channel band of oT: only that band (both query halves) is
  needed for this core's 2048 output rows.  After each query chunk T the band
  the partner needs is sent through a small bf16 AllGather (4 chunked
  collectives overlap the remaining attention compute); partition-id
  predicated DMAs (cond=) select the bands into a contiguous z staging
  buffer, so the output projection reads plain contiguous rows.
  out     2048 permuted rows @ Wp + bp (f32)
"""
from contextlib import ExitStack

import numpy as np

import concourse.bass as bass
import concourse.tile as tile
from concourse import bacc, mybir
from concourse.bass_utils import run_bass_kernel_spmd
from concourse.masks import make_identity

F32 = mybir.dt.float32
BF16 = mybir.dt.bfloat16
F32R = mybir.dt.float32r
EXP = mybir.ActivationFunctionType.Exp

B, N, TN, C = 4, 4096, 4096, 384
TNS = TN // 2
SCALE = (C // 8) ** -0.5
N_CORES = 8
CB = C // 2  # 192: channel band per core in the output permutation
NT = TNS // 512  # 4 query chunks per core

QK_DT = F32R   # q/k/score path (tf32-class)
VE_DT = BF16   # v/e/o path


def build(qk_dt=QK_DT, ve_dt=VE_DT, repeat=1, with_collective=True):
    nc = bacc.Bacc("TRN2", target_bir_lowering=False, debug=False,
                   num_devices=N_CORES)
    x_d = nc.dram_tensor("x", [N, C], F32, kind="ExternalInput").ap()
    t_d = nc.dram_tensor("t", [TNS, C], F32, kind="ExternalInput").ap()
    w_d = {n: nc.dram_tensor(n, [C, C], F32, kind="ExternalInput").ap()
           for n in ("Wq", "Wk", "Wv", "Wp")}
    bp_d = nc.dram_tensor("bp", [1, C], F32, kind="ExternalInput").ap()
    out_d = nc.dram_tensor("out", [TNS, C], F32, kind="ExternalOutput").ap()

    with tile.TileContext(nc) as tc:
        _kernel_body(nc, tc, x_d, t_d, w_d, bp_d, out_d, qk_dt, ve_dt,
                     repeat, with_collective)
    nc.compile()
    return nc


def _kernel_body(nc, tc, x_d, t_d, w_d, bp_d, out_d, qk_st, ve_st,
                 repeat, with_collective):
    with ExitStack() as ctx:
        consts = ctx.enter_context(tc.tile_pool(name="consts", bufs=1))
        persist = ctx.enter_context(tc.tile_pool(name="persist", bufs=1))
        dram = ctx.enter_context(tc.tile_pool(name="dram", bufs=1, space="DRAM"))

        ident_f = consts.tile([128, 128], F32)
        make_identity(nc, ident_f)
        ident_b = consts.tile([128, 128], ve_st)
        make_identity(nc, ident_b)
        ones_col = consts.tile([128, 1], ve_st)
        nc.vector.memset(ones_col[:], 1.0)
        ones_row_f = consts.tile([1, 128], F32)
        nc.vector.memset(ones_row_f[:], 1.0)
        ones_row = consts.tile([1, 128], qk_st)
        nc.vector.tensor_copy(ones_row[:], ones_row_f[:])

        # partition id -> query-half h, for the post-collective band select
        h_rv = nc.partition_id() % 2
        cond_h = ((h_rv + 1) % 2, h_rv)  # cond_h[hh] is truthy iff h == hh

        w_sb = {}
        with tc.tile_pool(name="wstage", bufs=2) as wstage:
            for name in ("Wq", "Wk", "Wv", "Wp"):
                cw = persist.tile([128, 3 * C], qk_st, name=f"{name}_sb",
                                  tag=f"{name}_sb")
                for dc in range(3):
                    st = wstage.tile([128, C], F32, name="wst", tag="wst")
                    nc.sync.dma_start(st[:], w_d[name][dc * 128:(dc + 1) * 128, :])
                    nc.scalar.copy(cw[:, dc * C:(dc + 1) * C], st[:])
                w_sb[name] = cw
            bst = wstage.tile([1, C], F32, name="bst", tag="wst")
            nc.sync.dma_start(bst[:], bp_d[:])
            with tc.tile_pool(name="bpsum", bufs=1, space="PSUM") as bpsum:
                bias_ps = bpsum.tile([128, C], F32)
                nc.tensor.matmul(bias_ps[:], ones_row_f[:], bst[:],
                                 start=True, stop=True)
                bias_b = persist.tile([128, C], F32)
                nc.vector.tensor_copy(bias_b[:], bias_ps[:])

        def wch(name, dc, cc=None):
            if cc is None:
                return w_sb[name][:, dc * C:(dc + 1) * C]
            return w_sb[name][:, dc * C + cc * 128: dc * C + (cc + 1) * 128]

        for rep in range(repeat):
            _one_pass(nc, tc, x_d, t_d, out_d, qk_st, ve_st, ident_f, ident_b,
                      ones_col, ones_row, wch, bias_b, dram, with_collective,
                      rep, cond_h)


def _band_spans(oT, hh, col0, col1):
    """SBUF sources covering oT channel band [hh*CB, (hh+1)*CB), cols
    [col0:col1), as (rows_in_band_slice, source_ap) pairs."""
    if hh == 0:
        return [((0, 128), oT[0][:, col0:col1]),
                ((128, 192), oT[1][0:64, col0:col1])]
    return [((0, 64), oT[1][64:128, col0:col1]),
            ((64, 192), oT[2][:, col0:col1])]


def _one_pass(nc, tc, x_d, t_d, out_d, qk_st, ve_st, ident_f, ident_b,
              ones_col, ones_row, wch, bias_b, dram, with_collective, rep,
              cond_h):
    # z staging: this core's 2048 permuted output rows, contiguous
    zh = dram.tile([TNS, C], ve_st, name=f"zh{rep}", tag="zh")
    zview = zh[:].rearrange("a b -> (a b)").rearrange("(c t) -> c t", t=TN)

    with tc.tile_pool(name="attin", bufs=1) as attin:
        # ---- t -> tT chunks -> qT (per-T tiles, channel-major) ----
        qT = [attin.tile([128, 3 * 512], qk_st, name=f"qT{T}", tag=f"qT{T}")
              for T in range(NT)]
        with tc.tile_pool(name="tstage", bufs=2) as tstage, \
             tc.tile_pool(name="trpsum", bufs=2, space="PSUM") as trpsum, \
             tc.tile_pool(name="qpsum", bufs=2, space="PSUM") as qpsum:
            for T in range(NT):
                tT = [tstage.tile([128, 512], qk_st, name=f"tT{dc}",
                                  tag=f"tT{dc}") for dc in range(3)]
                pst = [trpsum.tile([128, 512], F32, name=f"pst{dc}",
                                   tag=f"pst{dc}") for dc in range(3)]
                for i in range(4):
                    trow = tstage.tile([128, C], F32, name="trow", tag="trow")
                    nc.sync.dma_start(
                        trow[:], t_d[(T * 4 + i) * 128:(T * 4 + i + 1) * 128, :])
                    for dc in range(3):
                        nc.tensor.transpose(
                            pst[dc][:, i * 128:(i + 1) * 128],
                            trow[:, dc * 128:(dc + 1) * 128], ident_f[:])
                for dc in range(3):
                    nc.any.tensor_copy(tT[dc][:], pst[dc][:])
                for cc in range(3):
                    ps = qpsum.tile([128, 512], F32, name="qps", tag="qps")
                    for dc in range(3):
                        nc.tensor.matmul(ps[:], wch("Wq", dc, cc), tT[dc][:],
                                         start=(dc == 0), stop=(dc == 2))
                    nc.any.tensor_copy(qT[T][:, cc * 512:(cc + 1) * 512], ps[:])

        # ---- x -> xT chunks -> kT chunks + v chunks ----
        NCH = 4                      # key chunks of 1024
        KPC = N // NCH // 128        # 8 key blocks per chunk
        kT = [[attin.tile([128, 1024], qk_st, name=f"kT{cc}_{ch}",
                          tag=f"kT{cc}_{ch}") for ch in range(NCH)]
              for cc in range(3)]
        v = [attin.tile([128, KPC * C], ve_st, name=f"v{ch}", tag=f"v{ch}")
             for ch in range(NCH)]
        with tc.tile_pool(name="xstage", bufs=2) as xstage, \
             tc.tile_pool(name="xtrpsum", bufs=2, space="PSUM") as xtrpsum, \
             tc.tile_pool(name="kvpsum", bufs=2, space="PSUM") as kvpsum:
            for ch in range(NCH):
                xT = [xstage.tile([128, 1024], qk_st, name=f"xT{dc}",
                                  tag=f"xT{dc}") for dc in range(3)]
                for half in range(2):
                    pst = [xtrpsum.tile([128, 512], F32, name=f"xpst{dc}",
                                        tag=f"xpst{dc}") for dc in range(3)]
                    for i in range(4):
                        blk = ch * KPC + half * 4 + i
                        xrow = xstage.tile([128, C], F32, name="xrow",
                                           tag="xrow")
                        nc.sync.dma_start(
                            xrow[:], x_d[blk * 128:(blk + 1) * 128, :])
                        for dc in range(3):
                            nc.tensor.transpose(
                                pst[dc][:, i * 128:(i + 1) * 128],
                                xrow[:, dc * 128:(dc + 1) * 128], ident_f[:])
                    for dc in range(3):
                        nc.any.tensor_copy(
                            xT[dc][:, half * 512:(half + 1) * 512], pst[dc][:])
                for cc in range(3):
                    for nt in range(2):
                        ps = kvpsum.tile([128, 512], F32, name="kps", tag="kps")
                        for dc in range(3):
                            nc.tensor.matmul(
                                ps[:], wch("Wk", dc, cc),
                                xT[dc][:, nt * 512:(nt + 1) * 512],
                                start=(dc == 0), stop=(dc == 2))
                        nc.any.tensor_copy(
                            kT[cc][ch][:, nt * 512:(nt + 1) * 512], ps[:])
                for j in range(KPC):
                    ps = kvpsum.tile([128, C], F32, name="vps", tag="kps")
                    for dc in range(3):
                        nc.tensor.matmul(
                            ps[:], xT[dc][:, j * 128:(j + 1) * 128],
                            wch("Wv", dc), start=(dc == 0), stop=(dc == 2))
                    nc.any.tensor_copy(v[ch][:, j * C:(j + 1) * C], ps[:])

        # ---- attention + per-T chunked pair exchange ----
        oT = [attin.tile([128, TNS], ve_st, name=f"oT{cc}", tag=f"oT{cc}")
              for cc in range(3)]
        with tc.tile_pool(name="spsum", bufs=2, space="PSUM") as spsum, \
             tc.tile_pool(name="opsum", bufs=1, space="PSUM") as opsum, \
             tc.tile_pool(name="dpsum", bufs=1, space="PSUM") as dpsum, \
             tc.tile_pool(name="epool", bufs=3) as epool, \
             tc.tile_pool(name="npool", bufs=2) as npool:
            for T in range(NT):
                o_ps = [opsum.tile([128, 512], F32, name=f"ops{cc}",
                                   tag=f"ops{cc}") for cc in range(3)]
                d_ps = dpsum.tile([1, 512], F32, name="dps", tag="dps")
                qs = qT[T]
                for j in range(16):
                    s2 = spsum.tile([128, 1024], F32, name="sps", tag="sps")
                    for s in range(2):
                        n32 = 2 * j + s
                        ch, jj = n32 // KPC, n32 % KPC
                        for cc in range(3):
                            nc.tensor.matmul(
                                s2[:, s * 512:(s + 1) * 512],
                                kT[cc][ch][:, jj * 128:(jj + 1) * 128],
                                qs[:, cc * 512:(cc + 1) * 512],
                                start=(cc == 0), stop=(cc == 2))
                    e2 = epool.tile([128, 1024], ve_st, name="e2", tag="e2")
                    nc.scalar.activation(e2[:], s2[:], EXP, scale=SCALE)
                    for s in range(2):
                        n32 = 2 * j + s
                        ch, jj = n32 // KPC, n32 % KPC
                        es = e2[:, s * 512:(s + 1) * 512]
                        for cc in range(3):
                            nc.tensor.matmul(
                                o_ps[cc][:],
                                v[ch][:, jj * C + cc * 128: jj * C + (cc + 1) * 128],
                                es, start=(n32 == 0), stop=(n32 == 31))
                        nc.tensor.matmul(d_ps[:], ones_col[:], es,
                                         start=(n32 == 0), stop=(n32 == 31))
                rec = npool.tile([1, 512], qk_st, name="rec", tag="rec")
                with nc.allow_low_precision(
                        reason="f32r stores full f32 bits; PE truncation of "
                               "1/D to tf32 is within tolerance"):
                    nc.vector.reciprocal(rec[:], d_ps[:])
                b_ps = spsum.tile([128, 512], F32, name="bps", tag="sps")
                nc.tensor.matmul(b_ps[:], ones_row[:], rec[:],
                                 start=True, stop=True)
                rec_b = npool.tile([128, 512], F32, name="rec_b", tag="rec_b")
                nc.vector.tensor_copy(rec_b[:], b_ps[:])
                for cc in range(3):
                    nc.vector.tensor_mul(oT[cc][:, T * 512:(T + 1) * 512],
                                         o_ps[cc][:], rec_b[:])

                # pair exchange for this T chunk (overlaps later T compute)
                col0, col1 = T * 512, (T + 1) * 512
                if with_collective:
                    send = dram.tile([CB, 512], ve_st, name=f"send{rep}_{T}",
                                     tag=f"sendT{T}")
                    gath = dram.tile([2 * CB, 512], ve_st,
                                     name=f"gath{rep}_{T}", tag=f"gathT{T}")
                    for hh in range(2):
                        # send the band the partner (query-half 1-hh) needs
                        for (r0, r1), src in _band_spans(oT, 1 - hh, col0, col1):
                            nc.sync.dma_start(send[r0:r1, :], src,
                                              cond=cond_h[hh])
                        # own band, own query half -> z staging
                        for (r0, r1), src in _band_spans(oT, hh, col0, col1):
                            nc.sync.dma_start(
                                zview[r0:r1, hh * TNS + col0: hh * TNS + col1],
                                src, cond=cond_h[hh])
                    nc.gpsimd.collective_compute(
                        "AllGather", mybir.AluOpType.bypass,
                        replica_groups=[[0, 1], [2, 3], [4, 5], [6, 7]],
                        ins=[send[:].opt()], outs=[gath[:].opt()])
                    for hh in range(2):
                        # partner's contribution is their send = gath block 1-hh
                        g = 1 - hh
                        nc.sync.dma_start(
                            zview[:, g * TNS + col0: g * TNS + col1],
                            gath[g * CB:(g + 1) * CB, :], cond=cond_h[hh])
                else:
                    for hh in range(2):
                        for g in range(2):
                            for (r0, r1), src in _band_spans(oT, hh, col0, col1):
                                nc.sync.dma_start(
                                    zview[r0:r1,
                                          g * TNS + col0: g * TNS + col1],
                                    src, cond=cond_h[hh])

    # ---- permuted output projection (this core's 2048 rows only) ----
    with tc.tile_pool(name="fpool", bufs=3) as fpool, \
         tc.tile_pool(name="fpsum", bufs=2, space="PSUM") as fpsum, \
         tc.tile_pool(name="ftpsum", bufs=2, space="PSUM") as ftpsum:
        for it in range(TNS // 128):
            r_t = fpool.tile([128, C], ve_st, name="r_t", tag="r_t")
            nc.sync.dma_start(r_t[:], zh[it * 128:(it + 1) * 128, :])
            p_tr = ftpsum.tile([128, C], ve_st, name="p_tr", tag="p_tr")
            for jc in range(3):
                nc.tensor.transpose(p_tr[:, jc * 128:(jc + 1) * 128],
                                    r_t[:, jc * 128:(jc + 1) * 128],
                                    ident_b[:])
            op_ch = fpool.tile([128, 3 * 128], qk_st, name="op_ch", tag="op_ch")
            nc.any.tensor_copy(op_ch[:], p_tr[:])
            out_ps = fpsum.tile([128, C], F32, name="out_ps", tag="out_ps")
            for jc in range(3):
                nc.tensor.matmul(out_ps[:], op_ch[:, jc * 128:(jc + 1) * 128],
                                 wch("Wp", jc), start=(jc == 0), stop=(jc == 2))
            o_t = fpool.tile([128, C], F32, name="o_t", tag="o_t")
            nc.vector.tensor_add(o_t[:], out_ps[:], bias_b[:])
            nc.sync.dma_start(out_d[it * 128:(it + 1) * 128, :], o_t[:])


def make_in_maps(inputs):
    x = np.asarray(inputs["x"], np.float32)
    t = np.asarray(inputs["t"], np.float32)
    maps = []
    for p in range(N_CORES):
        b, h = p // 2, p % 2
        maps.append({
            "x": np.ascontiguousarray(x[b]),
            "t": np.ascontiguousarray(t[b, h * TNS:(h + 1) * TNS]),
            "Wq": np.asarray(inputs["Wq"], np.float32),
            "Wk": np.asarray(inputs["Wk"], np.float32),
            "Wv": np.asarray(inputs["Wv"], np.float32),
            "Wp": np.asarray(inputs["Wp"], np.float32),
            "bp": np.asarray(inputs["bp"], np.float32).reshape(1, C),
        })
    return maps


def assemble(results):
    out = np.empty((B, TN, C), np.float32)
    for p in range(N_CORES):
        b, h = p // 2, p % 2
        out[b, h * TNS:(h + 1) * TNS] = results[p]["out"]
    return out


_NC_CACHE = {}


def _get_nc(repeat=1):
    key = repeat
    if key not in _NC_CACHE:
        _NC_CACHE[key] = build(repeat=repeat)
    return _NC_CACHE[key]


def kernel(**inputs) -> np.ndarray:
    nc = _get_nc()
    in_maps = make_in_maps(inputs)
    res = run_bass_kernel_spmd(nc, in_maps, list(range(N_CORES)))
    return assemble(res.results)


# revision 11
# speedup vs baseline: 1.4881x; 1.4881x over previous
"""nn_CrossAttention Trainium2 kernel — 8-core SPMD Bass/Tile implementation.

Sharding: core p -> batch b = p//2, query-row half h = p%2 (data parallel over
B=4, sequence-parallel over TN within each batch pair).

Per-core dataflow (v3):
  tT,xT   PE-transpose of the inputs; four 128x128 transposes share one
          [128,512] PSUM tile so a single wide copy drains them (4x fewer
          PSUM->SBUF copies)
  qT,kT   projections in channel-major layout (f32r = tf32-class precision)
  v       projection in natural row-major layout (bf16)
  sT      score tiles computed *transposed* (keys on partitions, queries free)
          two key-blocks share one [128,1024] PSUM tile so a single 1024-wide
          ScalarE exp serves both (amortizes the +352-cycle ACT overhead)
  D       softmax denominators via ones-vector matmul (PE partition reduce)
  oT      v.T @ eT accumulated over keys, normalized by 1/D broadcast, bf16
  The reference's "transpose(1,2).reshape" permutation assigns each core a
  contiguous 192-channel band of oT: only that band (both query halves) is
  needed for this core's 2048 output rows.  After each query chunk T the band
  the partner needs is sent through a small bf16 AllGather (4 chunked
  collectives overlap the remaining attention compute); partition-id
  predicated DMAs (cond=) select the bands into a contiguous z staging
  buffer, so the output projection reads plain contiguous rows.
  out     2048 permuted rows @ Wp + bp (f32)
"""
from contextlib import ExitStack

import numpy as np

import concourse.bass as bass
import concourse.tile as tile
from concourse import bacc, mybir
from concourse.bass_utils import run_bass_kernel_spmd
from concourse.masks import make_identity

F32 = mybir.dt.float32
BF16 = mybir.dt.bfloat16
F32R = mybir.dt.float32r
EXP = mybir.ActivationFunctionType.Exp

B, N, TN, C = 4, 4096, 4096, 384
TNS = TN // 2
SCALE = (C // 8) ** -0.5
N_CORES = 8
CB = C // 2  # 192: channel band per core in the output permutation
NT = TNS // 512  # 4 query chunks per core

QK_DT = F32R   # q/k/score path (tf32-class)
VE_DT = BF16   # v/e/o path


def build(qk_dt=QK_DT, ve_dt=VE_DT, repeat=1, with_collective=True):
    nc = bacc.Bacc("TRN2", target_bir_lowering=False, debug=False,
                   num_devices=N_CORES)
    x_d = nc.dram_tensor("x", [N, C], F32, kind="ExternalInput").ap()
    t_d = nc.dram_tensor("t", [TNS, C], F32, kind="ExternalInput").ap()
    w_d = {n: nc.dram_tensor(n, [C, C], F32, kind="ExternalInput").ap()
           for n in ("Wq", "Wk", "Wv", "Wp")}
    bp_d = nc.dram_tensor("bp", [1, C], F32, kind="ExternalInput").ap()
    out_d = nc.dram_tensor("out", [TNS, C], F32, kind="ExternalOutput").ap()

    with tile.TileContext(nc) as tc:
        _kernel_body(nc, tc, x_d, t_d, w_d, bp_d, out_d, qk_dt, ve_dt,
                     repeat, with_collective)
    nc.compile()
    return nc


def _kernel_body(nc, tc, x_d, t_d, w_d, bp_d, out_d, qk_st, ve_st,
                 repeat, with_collective):
    with ExitStack() as ctx:
        consts = ctx.enter_context(tc.tile_pool(name="consts", bufs=1))
        persist = ctx.enter_context(tc.tile_pool(name="persist", bufs=1))
        dram = ctx.enter_context(tc.tile_pool(name="dram", bufs=1, space="DRAM"))

        ident_f = consts.tile([128, 128], F32)
        make_identity(nc, ident_f)
        ident_b = consts.tile([128, 128], ve_st)
        make_identity(nc, ident_b)
        ones_col = consts.tile([128, 1], ve_st)
        nc.vector.memset(ones_col[:], 1.0)
        ones_row_f = consts.tile([1, 128], F32)
        nc.vector.memset(ones_row_f[:], 1.0)
        ones_row = consts.tile([1, 128], qk_st)
        nc.vector.tensor_copy(ones_row[:], ones_row_f[:])

        # partition id -> query-half h, for the post-collective band select
        h_rv = nc.partition_id() % 2
        cond_h = ((h_rv + 1) % 2, h_rv)  # cond_h[hh] is truthy iff h == hh

        w_sb = {}
        with tc.tile_pool(name="wstage", bufs=2) as wstage:
            for name in ("Wq", "Wk", "Wv", "Wp"):
                cw = persist.tile([128, 3 * C], qk_st, name=f"{name}_sb",
                                  tag=f"{name}_sb")
                for dc in range(3):
                    st = wstage.tile([128, C], F32, name="wst", tag="wst")
                    nc.sync.dma_start(st[:], w_d[name][dc * 128:(dc + 1) * 128, :])
                    nc.scalar.copy(cw[:, dc * C:(dc + 1) * C], st[:])
                w_sb[name] = cw
            bst = wstage.tile([1, C], F32, name="bst", tag="wst")
            nc.sync.dma_start(bst[:], bp_d[:])
            with tc.tile_pool(name="bpsum", bufs=1, space="PSUM") as bpsum:
                bias_ps = bpsum.tile([128, C], F32)
                nc.tensor.matmul(bias_ps[:], ones_row_f[:], bst[:],
                                 start=True, stop=True)
                bias_b = persist.tile([128, C], F32)
                nc.vector.tensor_copy(bias_b[:], bias_ps[:])

        def wch(name, dc, cc=None):
            if cc is None:
                return w_sb[name][:, dc * C:(dc + 1) * C]
            return w_sb[name][:, dc * C + cc * 128: dc * C + (cc + 1) * 128]

        for rep in range(repeat):
            _one_pass(nc, tc, x_d, t_d, out_d, qk_st, ve_st, ident_f, ident_b,
                      ones_col, ones_row, wch, bias_b, dram, with_collective,
                      rep, cond_h)


def _band_spans(oT, hh, col0, col1):
    """SBUF sources covering oT channel band [hh*CB, (hh+1)*CB), cols
    [col0:col1), as (rows_in_band_slice, source_ap) pairs."""
    if hh == 0:
        return [((0, 128), oT[0][:, col0:col1]),
                ((128, 192), oT[1][0:64, col0:col1])]
    return [((0, 64), oT[1][64:128, col0:col1]),
            ((64, 192), oT[2][:, col0:col1])]


def _one_pass(nc, tc, x_d, t_d, out_d, qk_st, ve_st, ident_f, ident_b,
              ones_col, ones_row, wch, bias_b, dram, with_collective, rep,
              cond_h):
    # z staging: this core's 2048 permuted output rows, contiguous
    zh = dram.tile([TNS, C], ve_st, name=f"zh{rep}", tag=f"zh{rep % 2}")
    zview = zh[:].rearrange("a b -> (a b)").rearrange("(c t) -> c t", t=TN)

    with tc.tile_pool(name="attin", bufs=1) as attin:
        # ---- t -> tT chunks -> qT (per-T tiles, channel-major) ----
        qT = [attin.tile([128, 3 * 512], qk_st, name=f"qT{T}", tag=f"qT{T}")
              for T in range(NT)]
        with tc.tile_pool(name="tstage", bufs=4) as tstage, \
             tc.tile_pool(name="trpsum", bufs=2, space="PSUM") as trpsum, \
             tc.tile_pool(name="qpsum", bufs=2, space="PSUM") as qpsum:
            for T in range(NT):
                tT = [tstage.tile([128, 512], qk_st, name=f"tT{dc}",
                                  tag=f"tT{dc}") for dc in range(3)]
                pst = [trpsum.tile([128, 512], F32, name=f"pst{dc}",
                                   tag=f"pst{dc}") for dc in range(3)]
                for i in range(4):
                    trow = tstage.tile([128, C], F32, name="trow", tag="trow")
                    nc.sync.dma_start(
                        trow[:], t_d[(T * 4 + i) * 128:(T * 4 + i + 1) * 128, :])
                    for dc in range(3):
                        nc.tensor.transpose(
                            pst[dc][:, i * 128:(i + 1) * 128],
                            trow[:, dc * 128:(dc + 1) * 128], ident_f[:])
                for dc in range(3):
                    nc.any.tensor_copy(tT[dc][:], pst[dc][:])
                for cc in range(3):
                    ps = qpsum.tile([128, 512], F32, name="qps", tag="qps")
                    for dc in range(3):
                        nc.tensor.matmul(ps[:], wch("Wq", dc, cc), tT[dc][:],
                                         start=(dc == 0), stop=(dc == 2))
                    nc.any.tensor_copy(qT[T][:, cc * 512:(cc + 1) * 512], ps[:])

        # ---- x -> xT chunks -> kT chunks + v chunks ----
        NCH = 4                      # key chunks of 1024
        KPC = N // NCH // 128        # 8 key blocks per chunk
        kT = [[attin.tile([128, 1024], qk_st, name=f"kT{cc}_{ch}",
                          tag=f"kT{cc}_{ch}") for ch in range(NCH)]
              for cc in range(3)]
        v = [attin.tile([128, KPC * C], ve_st, name=f"v{ch}", tag=f"v{ch}")
             for ch in range(NCH)]
        with tc.tile_pool(name="xstage", bufs=4) as xstage, \
             tc.tile_pool(name="xtrpsum", bufs=2, space="PSUM") as xtrpsum, \
             tc.tile_pool(name="kvpsum", bufs=2, space="PSUM") as kvpsum:
            for ch in range(NCH):
                xT = [xstage.tile([128, 1024], qk_st, name=f"xT{dc}",
                                  tag=f"xT{dc}") for dc in range(3)]
                for half in range(2):
                    pst = [xtrpsum.tile([128, 512], F32, name=f"xpst{dc}",
                                        tag=f"xpst{dc}") for dc in range(3)]
                    for i in range(4):
                        blk = ch * KPC + half * 4 + i
                        xrow = xstage.tile([128, C], F32, name="xrow",
                                           tag="xrow")
                        nc.sync.dma_start(
                            xrow[:], x_d[blk * 128:(blk + 1) * 128, :])
                        for dc in range(3):
                            nc.tensor.transpose(
                                pst[dc][:, i * 128:(i + 1) * 128],
                                xrow[:, dc * 128:(dc + 1) * 128], ident_f[:])
                    for dc in range(3):
                        nc.any.tensor_copy(
                            xT[dc][:, half * 512:(half + 1) * 512], pst[dc][:])
                for cc in range(3):
                    for nt in range(2):
                        ps = kvpsum.tile([128, 512], F32, name="kps", tag="kps")
                        for dc in range(3):
                            nc.tensor.matmul(
                                ps[:], wch("Wk", dc, cc),
                                xT[dc][:, nt * 512:(nt + 1) * 512],
                                start=(dc == 0), stop=(dc == 2))
                        nc.any.tensor_copy(
                            kT[cc][ch][:, nt * 512:(nt + 1) * 512], ps[:])
                for j in range(KPC):
                    ps = kvpsum.tile([128, C], F32, name="vps", tag="kps")
                    for dc in range(3):
                        nc.tensor.matmul(
                            ps[:], xT[dc][:, j * 128:(j + 1) * 128],
                            wch("Wv", dc), start=(dc == 0), stop=(dc == 2))
                    nc.any.tensor_copy(v[ch][:, j * C:(j + 1) * C], ps[:])

        # ---- attention + per-T chunked pair exchange ----
        oT = [attin.tile([128, TNS], ve_st, name=f"oT{cc}", tag=f"oT{cc}")
              for cc in range(3)]
        with tc.tile_pool(name="spsum", bufs=2, space="PSUM") as spsum, \
             tc.tile_pool(name="opsum", bufs=1, space="PSUM") as opsum, \
             tc.tile_pool(name="dpsum", bufs=1, space="PSUM") as dpsum, \
             tc.tile_pool(name="epool", bufs=3) as epool, \
             tc.tile_pool(name="npool", bufs=2) as npool:
            for T in range(NT):
                o_ps = [opsum.tile([128, 512], F32, name=f"ops{cc}",
                                   tag=f"ops{cc}") for cc in range(3)]
                d_ps = dpsum.tile([1, 512], F32, name="dps", tag="dps")
                qs = qT[T]
                for j in range(16):
                    s2 = spsum.tile([128, 1024], F32, name="sps", tag="sps")
                    for s in range(2):
                        n32 = 2 * j + s
                        ch, jj = n32 // KPC, n32 % KPC
                        for cc in range(3):
                            nc.tensor.matmul(
                                s2[:, s * 512:(s + 1) * 512],
                                kT[cc][ch][:, jj * 128:(jj + 1) * 128],
                                qs[:, cc * 512:(cc + 1) * 512],
                                start=(cc == 0), stop=(cc == 2))
                    e2 = epool.tile([128, 1024], ve_st, name="e2", tag="e2")
                    nc.scalar.activation(e2[:], s2[:], EXP, scale=SCALE)
                    for s in range(2):
                        n32 = 2 * j + s
                        ch, jj = n32 // KPC, n32 % KPC
                        es = e2[:, s * 512:(s + 1) * 512]
                        for cc in range(3):
                            nc.tensor.matmul(
                                o_ps[cc][:],
                                v[ch][:, jj * C + cc * 128: jj * C + (cc + 1) * 128],
                                es, start=(n32 == 0), stop=(n32 == 31))
                        nc.tensor.matmul(d_ps[:], ones_col[:], es,
                                         start=(n32 == 0), stop=(n32 == 31))
                rec = npool.tile([1, 512], qk_st, name="rec", tag="rec")
                with nc.allow_low_precision(
                        reason="f32r stores full f32 bits; PE truncation of "
                               "1/D to tf32 is within tolerance"):
                    nc.vector.reciprocal(rec[:], d_ps[:])
                b_ps = spsum.tile([128, 512], F32, name="bps", tag="sps")
                nc.tensor.matmul(b_ps[:], ones_row[:], rec[:],
                                 start=True, stop=True)
                rec_b = npool.tile([128, 512], F32, name="rec_b", tag="rec_b")
                nc.vector.tensor_copy(rec_b[:], b_ps[:])
                for cc in range(3):
                    nc.vector.tensor_mul(oT[cc][:, T * 512:(T + 1) * 512],
                                         o_ps[cc][:], rec_b[:])

                # pair exchange for this T chunk (overlaps later T compute)
                col0, col1 = T * 512, (T + 1) * 512
                if with_collective:
                    send = dram.tile([CB, 512], ve_st, name=f"send{rep}_{T}",
                                     tag=f"sendT{T}")
                    gath = dram.tile([2 * CB, 512], ve_st,
                                     name=f"gath{rep}_{T}", tag=f"gathT{T}")
                    for hh in range(2):
                        # send the band the partner (query-half 1-hh) needs
                        for (r0, r1), src in _band_spans(oT, 1 - hh, col0, col1):
                            nc.sync.dma_start(send[r0:r1, :], src,
                                              cond=cond_h[hh])
                        # own band, own query half -> z staging
                        for (r0, r1), src in _band_spans(oT, hh, col0, col1):
                            nc.sync.dma_start(
                                zview[r0:r1, hh * TNS + col0: hh * TNS + col1],
                                src, cond=cond_h[hh])
                    nc.gpsimd.collective_compute(
                        "AllGather", mybir.AluOpType.bypass,
                        replica_groups=[[0, 1], [2, 3], [4, 5], [6, 7]],
                        ins=[send[:].opt()], outs=[gath[:].opt()])
                    for hh in range(2):
                        # partner's contribution is their send = gath block 1-hh
                        g = 1 - hh
                        nc.sync.dma_start(
                            zview[:, g * TNS + col0: g * TNS + col1],
                            gath[g * CB:(g + 1) * CB, :], cond=cond_h[hh])
                else:
                    for hh in range(2):
                        for g in range(2):
                            for (r0, r1), src in _band_spans(oT, hh, col0, col1):
                                nc.sync.dma_start(
                                    zview[r0:r1,
                                          g * TNS + col0: g * TNS + col1],
                                    src, cond=cond_h[hh])

    # ---- permuted output projection (this core's 2048 rows only) ----
    with tc.tile_pool(name="fpool", bufs=3) as fpool, \
         tc.tile_pool(name="fpsum", bufs=2, space="PSUM") as fpsum, \
         tc.tile_pool(name="ftpsum", bufs=2, space="PSUM") as ftpsum:
        for it in range(TNS // 128):
            r_t = fpool.tile([128, C], ve_st, name="r_t", tag="r_t")
            nc.sync.dma_start(r_t[:], zh[it * 128:(it + 1) * 128, :])
            p_tr = ftpsum.tile([128, C], ve_st, name="p_tr", tag="p_tr")
            for jc in range(3):
                nc.tensor.transpose(p_tr[:, jc * 128:(jc + 1) * 128],
                                    r_t[:, jc * 128:(jc + 1) * 128],
                                    ident_b[:])
            op_ch = fpool.tile([128, 3 * 128], qk_st, name="op_ch", tag="op_ch")
            nc.any.tensor_copy(op_ch[:], p_tr[:])
            out_ps = fpsum.tile([128, C], F32, name="out_ps", tag="out_ps")
            for jc in range(3):
                nc.tensor.matmul(out_ps[:], op_ch[:, jc * 128:(jc + 1) * 128],
                                 wch("Wp", jc), start=(jc == 0), stop=(jc == 2))
            o_t = fpool.tile([128, C], F32, name="o_t", tag="o_t")
            nc.vector.tensor_add(o_t[:], out_ps[:], bias_b[:])
            nc.sync.dma_start(out_d[it * 128:(it + 1) * 128, :], o_t[:])


def make_in_maps(inputs):
    x = np.asarray(inputs["x"], np.float32)
    t = np.asarray(inputs["t"], np.float32)
    maps = []
    for p in range(N_CORES):
        b, h = p // 2, p % 2
        maps.append({
            "x": np.ascontiguousarray(x[b]),
            "t": np.ascontiguousarray(t[b, h * TNS:(h + 1) * TNS]),
            "Wq": np.asarray(inputs["Wq"], np.float32),
            "Wk": np.asarray(inputs["Wk"], np.float32),
            "Wv": np.asarray(inputs["Wv"], np.float32),
            "Wp": np.asarray(inputs["Wp"], np.float32),
            "bp": np.asarray(inputs["bp"], np.float32).reshape(1, C),
        })
    return maps


def assemble(results):
    out = np.empty((B, TN, C), np.float32)
    for p in range(N_CORES):
        b, h = p // 2, p % 2
        out[b, h * TNS:(h + 1) * TNS] = results[p]["out"]
    return out


_NC_CACHE = {}


def _get_nc(repeat=1):
    key = repeat
    if key not in _NC_CACHE:
        _NC_CACHE[key] = build(repeat=repeat)
    return _NC_CACHE[key]


def kernel(**inputs) -> np.ndarray:
    nc = _get_nc()
    in_maps = make_in_maps(inputs)
    res = run_bass_kernel_spmd(nc, in_maps, list(range(N_CORES)))
    return assemble(res.results)
